# revision 47
# baseline (speedup 1.0000x reference)
"""Trainium2 Bass kernel: batch-independent contrastive loss (SupCon-style with
EMA-normalized negatives).

Math (derived from the reference):
  CF = concat(views) [N=4096, D=256], S = CF @ CF.T / T, s_ij = f_i.f_j/T
  Each row i has exactly one positive p(i) = (i+B) mod N; neg_mask keeps the
  diagonal.  With m_i = ||f_i||^2/T:
    P_i  = sum_j exp(s_ij),  Q_i = sum_j exp(s_ij) s_ij
    Zneg_i = e^{-m_i} P_i - e_pos_i
    Wneg_i = e^{-m_i} (Q_i - m_i P_i) - e_pos_i Lpos_i
    u_new  = (1-g) u[idx] + g Zneg   (view-0 rows)
    loss_i = Wneg_i / u_new_{i mod B} - Lpos_i ;  output = mean_i loss_i

Key structural trick: with E'_ji = exp(s_ji - C + K) (C = 1/T ~ the common
row max since features are unit-norm, K = 8 a range shift), both P and Q are
matmul reductions over j:
    G_i[c]  = sum_j E'_ji f_j[c]   (c < 256)     -> Q_i = e^{C-K}/T f_i.G_i
    G_i[256]= sum_j E'_ji          (ones column) -> P_i = e^{C-K} G_i[256]
so the [N,N]-sized reduction work runs on the Tensor engine.

Per core (512 anchor rows i, all 4096 contrast j): 32 fp8 DoubleRow matmuls
build S^T in [128 j, 1024(=2x512 i)] PSUM pair-tiles; the Scalar engine does
exp (constant bias, no accumulate) into fp8-e5m2 E' tiles; 64 fp8 DoubleRow
G-matmuls accumulate G into 4 persistent PSUM banks; the Vector engine only
does tiny per-row stats, 4 [128,256] rowdots, and the final combine.  1/T is
folded into the anchor fp8 values on the host so PSUM holds s directly.
"""

import numpy as np
import ml_dtypes

GAMMA = 0.9
TEMP = 0.07
CB = 1.0 / TEMP      # common shift (row max for unit-norm features)
KS = 8.0             # range shift so e5m2 holds the mass
B, V, D = 2048, 2, 256
N = B * V            # 4096 contrast rows/cols
NCORES = 8
SPC = B // NCORES    # 256 samples per core
RPC = V * SPC        # 512 anchor rows per core
RC = RPC // 128      # 4 chunks of 128 anchor rows (0,1: view0; 2,3: view1)
NJC = N // 128       # 32 contrast chunks of 128
NJP = NJC // 2       # 16 chunk pairs
GW = 260             # G width: 256 features + ones col + pad

_CACHE = {}


def _build_module():
    import concourse.bacc as bacc
    import concourse.tile as tile
    from concourse import mybir

    f32 = mybir.dt.float32
    bf16 = mybir.dt.bfloat16
    f8e4 = mybir.dt.float8e4
    f8e5 = mybir.dt.float8e5
    AF = mybir.ActivationFunctionType
    ALU = mybir.AluOpType
    DR = mybir.MatmulPerfMode.DoubleRow

    nc = bacc.Bacc(
        "TRN2", target_bir_lowering=False, debug=False, enable_asserts=False
    )
    # head = anchor fp8 block + ct chunks 0-7, fused so the gating transfer
    # has 3KB per-partition runs (small runs move at ~20GB/s, big at ~100+)
    head_d = nc.dram_tensor("head8", [128, 2 * RPC + 2048], f8e4,
                            kind="ExternalInput")
    # ct chunk-major: [128 d-low, 32 chunks x (2 k x 128 cols)]
    ct8_d = nc.dram_tensor("ct8", [128, NJC * 256], f8e4, kind="ExternalInput")
    # F (G-matmul rhs): [128, jp-major x (2 halves x 260)] fp8
    f8_d = nc.dram_tensor("f8", [128, NJP * 2 * GW], f8e4, kind="ExternalInput")
    fa_d = nc.dram_tensor("fa", [128, RC * D], bf16, kind="ExternalInput")
    ug_d = nc.dram_tensor("ug", [128, 2], f32, kind="ExternalInput")
    out_d = nc.dram_tensor("loss_rows", [128, RC], f32, kind="ExternalOutput")

    with tile.TileContext(nc) as tc:
        with tc.tile_pool(name="singles", bufs=1) as singles, \
             tc.tile_pool(name="spsum", bufs=2, space="PSUM") as spsum, \
             tc.tile_pool(name="gpsum", bufs=1, space="PSUM") as gpsum, \
             tc.tile_pool(name="epool", bufs=6) as epool, \
             tc.tile_pool(name="work", bufs=2) as work, \
             tc.tile_pool(name="stats", bufs=1) as stats:
            # ---- DMA (2 HWDGE rings: sync + scalar; no gpsimd SWDGE so the
            # exit drain stays cheap).  ct q0 split so the first matmul can
            # start as soon as a small first bite lands.
            # Ring plan (measured per-engine rates: scalar ring ~8GB/s/e,
            # gpsimd ~4.5, sync ~1.5): the matmul-gating payloads ride the
            # scalar ring sized to their consumption deadline; mid/late ct
            # pieces ride gpsimd; only fa/ug (tail-only) ride sync.
            # head = anc8 + ct chunks 0-7 as ONE early transfer on the fast
            # scalar ring (single completion gates the whole front of the
            # pipeline); ct chunks 8-31 ride gpsimd; fa follows on scalar;
            # only tiny ug uses the slow sync ring.
            head_t = singles.tile([128, 2 * RPC + 2048], f8e4)
            nc.scalar.dma_start(out=head_t, in_=head_d[:, :])
            anc8_flat = head_t[:, 0:2 * RPC]
            ct_AB = head_t[:, 2 * RPC:2 * RPC + 2048]
            f8_a = singles.tile([128, 5 * 2 * GW], f8e4)
            f8_b = singles.tile([128, 6 * 2 * GW], f8e4)
            nc.gpsimd.dma_start(out=f8_a, in_=f8_d[:, 0:5 * 2 * GW])
            ct_C = singles.tile([128, 8 * 256], f8e4)
            nc.scalar.dma_start(out=ct_C, in_=ct8_d[:, 2048:4096])
            ct_D = singles.tile([128, 8 * 256], f8e4)
            nc.gpsimd.dma_start(out=ct_D, in_=ct8_d[:, 4096:6144])
            ct_E = singles.tile([128, 8 * 256], f8e4)
            nc.gpsimd.dma_start(out=ct_E, in_=ct8_d[:, 6144:8192])
            ug_sb = singles.tile([128, 2], f32)
            nc.sync.dma_start(out=ug_sb, in_=ug_d[:, :])
            # F pieces (jp 0-4, 5-10, 11-15); issues for b/c and fa are
            # interleaved into the scalar exp stream inside the main loop
            f8_c = singles.tile([128, 5 * 2 * GW], f8e4)
            fa_flat = singles.tile([128, RC * D], bf16)

            anc_v = anc8_flat.rearrange("p (k r) -> p k r", k=2)
            fa_sb = fa_flat.rearrange("p (rc d) -> p rc d", rc=RC)
            ct_pieces = [(0, ct_AB.rearrange("p (c k j) -> p c k j", c=8, k=2)),
                         (8, ct_C.rearrange("p (c k j) -> p c k j", c=8, k=2)),
                         (16, ct_D.rearrange("p (c k j) -> p c k j", c=8, k=2)),
                         (24, ct_E.rearrange("p (c k j) -> p c k j", c=8, k=2))]

            def ct_chunk(jc):
                # lhsT [128, 2, 128] for contrast chunk jc
                for base, view in reversed(ct_pieces):
                    if jc >= base:
                        return view[:, jc - base]

            f8_views = [(0, f8_a.rearrange("p (jp h c) -> p jp h c", jp=5, h=2)),
                        (5, f8_b.rearrange("p (jp h c) -> p jp h c", jp=6, h=2)),
                        (11, f8_c.rearrange("p (jp h c) -> p jp h c", jp=5, h=2))]

            def f8_pair(jp):
                # rhs [128, 2, GW] for chunk pair jp
                for base, view in reversed(f8_views):
                    if jp >= base:
                        return view[:, jp - base]

            # persistent G accumulators (4 x 1 PSUM bank)
            gps = []
            for ic in range(4):
                g_acc = gpsum.tile([128, GW], f32, tag=f"g{ic}", name=f"g{ic}")
                gps.append(g_acc)

            bias_p = stats.tile([128, 1], f32)  # +(C-K) for em2
            nc.vector.memset(bias_p, CB - KS)
            bias_n = stats.tile([128, 1], f32)  # -(C-K) for the main exp
            nc.vector.memset(bias_n, KS - CB)

            # ---- main loop over 16 contrast chunk-pairs ----
            # PE warmup happens naturally: first S-matmuls gate on the anc/ct
            # DMAs; emit a few dummy DR matmuls first to ramp the clock.
            warm_sb = singles.tile([128, 2 * 256], f8e4)
            nc.vector.memset(warm_sb, 0.0)
            warm_v = warm_sb.rearrange("p (k j) -> p k j", k=2)

            e_tiles = [None] * NJP

            def emit_g(jp):
                e8v = e_tiles[jp].rearrange("p (h i) -> p h i", h=2)
                for ic in range(4):
                    nc.tensor.matmul(
                        gps[ic],
                        lhsT=e8v[:, :, ic * 128:(ic + 1) * 128],
                        rhs=f8_pair(jp),
                        start=(jp == 0), stop=(jp == NJP - 1),
                        perf_mode=DR, skip_group_check=True,
                    )

            for jp in range(NJP):
                ps = spsum.tile([128, 1024], f32, tag="ps")
                if jp == 0:
                    for w in range(6):
                        nc.tensor.matmul(
                            ps[:, 0:256], lhsT=warm_v[:, :, 0:128],
                            rhs=warm_v, start=True, stop=True, perf_mode=DR)
                for h in range(2):
                    jc = 2 * jp + h
                    nc.tensor.matmul(
                        ps[:, h * 512:(h + 1) * 512],
                        lhsT=ct_chunk(jc),
                        rhs=anc_v,
                        start=True, stop=True, perf_mode=DR,
                    )
                e8 = epool.tile([128, 1024], f8e5, tag="e8")
                nc.scalar.activation(out=e8, in_=ps, func=AF.Exp,
                                     bias=bias_n)
                e_tiles[jp] = e8
                if jp == 2:
                    nc.scalar.dma_start(out=f8_b, in_=f8_d[:, 10 * GW:22 * GW])
                elif jp == 6:
                    nc.scalar.dma_start(out=f8_c, in_=f8_d[:, 22 * GW:32 * GW])
                elif jp == 8:
                    nc.scalar.dma_start(out=fa_flat, in_=fa_d[:, :])
                if jp >= 2:
                    emit_g(jp - 2)
            # ---- per-row statistics from the bf16 anchor features ----
            msum = stats.tile([128, RC], f32)   # ||f_r||^2
            for rc in range(RC):
                scr2 = work.tile([128, D], f32, tag="scr2")
                nc.vector.scalar_tensor_tensor(
                    out=scr2, in0=fa_sb[:, rc, :], scalar=1.0,
                    in1=fa_sb[:, rc, :], op0=ALU.mult, op1=ALU.mult,
                    accum_out=msum[:, rc:rc + 1],
                )
            pd = stats.tile([128, 2], f32)      # f_view0 . f_view1 per sample
            for s in range(2):
                scr2 = work.tile([128, D], f32, tag="scr2")
                nc.vector.scalar_tensor_tensor(
                    out=scr2, in0=fa_sb[:, s, :], scalar=1.0,
                    in1=fa_sb[:, 2 + s, :], op0=ALU.mult, op1=ALU.mult,
                    accum_out=pd[:, s:s + 1],
                )
            m4 = stats.tile([128, RC], f32)     # m = msum/T
            nc.vector.tensor_scalar_mul(m4, msum, 1.0 / TEMP)
            em2 = stats.tile([128, RC], f32)    # e^{(C-K) - m}
            nc.scalar.activation(out=em2, in_=msum, func=AF.Exp,
                                 scale=-1.0 / TEMP, bias=bias_p)
            pd4 = stats.tile([128, RC], f32)
            nc.vector.tensor_copy(pd4[:, 0:2], pd)
            nc.vector.tensor_copy(pd4[:, 2:4], pd)
            lp2 = stats.tile([128, RC], f32)    # Lpos = pd/T - m
            nc.vector.scalar_tensor_tensor(
                out=lp2, in0=pd4, scalar=1.0 / TEMP, in1=m4,
                op0=ALU.mult, op1=ALU.subtract)
            ep = stats.tile([128, RC], f32)     # e_pos
            nc.scalar.activation(out=ep, in_=lp2, func=AF.Exp)
            epl = stats.tile([128, RC], f32)
            nc.vector.tensor_mul(epl, ep, lp2)

            emit_g(NJP - 2)
            # final pair's G-matmuls interleaved with their rowdots so the
            # vector tail overlaps the last PE work
            qd4 = stats.tile([128, RC], f32)
            pacc4 = stats.tile([128, RC], f32)
            e8v_l = e_tiles[NJP - 1].rearrange("p (h i) -> p h i", h=2)
            for ic in range(4):
                nc.tensor.matmul(
                    gps[ic],
                    lhsT=e8v_l[:, :, ic * 128:(ic + 1) * 128],
                    rhs=f8_pair(NJP - 1),
                    start=False, stop=True,
                    perf_mode=DR, skip_group_check=True,
                )
                scr3 = work.tile([128, D], f32, tag="scr3")
                nc.vector.scalar_tensor_tensor(
                    out=scr3, in0=gps[ic][:, 0:D], scalar=1.0,
                    in1=fa_sb[:, ic, :], op0=ALU.mult, op1=ALU.mult,
                    accum_out=qd4[:, ic:ic + 1],
                )
                nc.vector.tensor_copy(pacc4[:, ic:ic + 1], gps[ic][:, D:D + 1])

            mp4 = stats.tile([128, RC], f32)    # m * pacc
            nc.vector.tensor_mul(mp4, m4, pacc4)
            w4 = stats.tile([128, RC], f32)     # qd4/T - m*pacc
            nc.vector.scalar_tensor_tensor(
                out=w4, in0=qd4, scalar=1.0 / TEMP, in1=mp4,
                op0=ALU.mult, op1=ALU.subtract)
            wem = stats.tile([128, RC], f32)
            nc.vector.tensor_mul(wem, em2, w4)
            wn = stats.tile([128, RC], f32)     # Wneg
            nc.vector.tensor_sub(wn, wem, epl)

            z2 = stats.tile([128, 2], f32)
            nc.vector.tensor_mul(z2, em2[:, 0:2], pacc4[:, 0:2])
            zn2 = stats.tile([128, 2], f32)
            nc.vector.tensor_sub(zn2, z2, ep[:, 0:2])
            un = stats.tile([128, 2], f32)
            nc.vector.scalar_tensor_tensor(
                out=un, in0=zn2, scalar=GAMMA, in1=ug_sb,
                op0=ALU.mult, op1=ALU.add)
            ru = stats.tile([128, 2], f32)
            nc.vector.reciprocal(ru, un)
            c4 = stats.tile([128, RC], f32)
            nc.vector.tensor_mul(c4[:, 0:2], wn[:, 0:2], ru)
            nc.vector.tensor_mul(c4[:, 2:4], wn[:, 2:4], ru)
            out_sb = stats.tile([128, RC], f32)
            nc.vector.tensor_sub(out_sb, c4, lp2)
            nc.scalar.dma_start(out=out_d[:, :], in_=out_sb)

    nc.compile()
    return nc


def _get_module():
    if "nc" not in _CACHE:
        _CACHE["nc"] = _build_module()
    return _CACHE["nc"]


def _prep_inputs(index, features, u):
    feats = np.asarray(features, dtype=np.float32)
    idx = np.asarray(index).astype(np.int64).reshape(-1)
    u_np = np.asarray(u, dtype=np.float32).reshape(-1)

    cf = np.ascontiguousarray(feats.transpose(1, 0, 2).reshape(N, D))
    cfb = cf.astype(ml_dtypes.bfloat16)
    ct = np.ascontiguousarray(cf.T)                        # [D, N] f32
    ct8 = ct.astype(ml_dtypes.float8_e4m3)
    # chunk-major [128, 32 chunks x (2 k x 128 cols)]
    ct_in = np.ascontiguousarray(
        ct8.reshape(2, 128, NJC, 128).transpose(1, 2, 0, 3)
        .reshape(128, NJC * 256))
    # F for the G-matmul: [128 j-low, jp, h, 260]
    f8 = np.zeros((128, NJP, 2, GW), dtype=ml_dtypes.float8_e4m3)
    cf8 = cf.astype(ml_dtypes.float8_e4m3)                 # [N, D]
    f8[:, :, :, 0:D] = cf8.reshape(NJP, 2, 128, D).transpose(2, 0, 1, 3)
    f8[:, :, :, D] = np.float32(1.0)
    f8_in = np.ascontiguousarray(f8.reshape(128, NJP * 2 * GW))

    in_maps = []
    for c in range(NCORES):
        rows = np.concatenate([
            np.arange(c * SPC, (c + 1) * SPC),
            np.arange(B + c * SPC, B + (c + 1) * SPC),
        ])
        anc_r = (ct[:, rows] / TEMP).astype(ml_dtypes.float8_e4m3)
        anc = np.empty((128, 2 * RPC), dtype=ml_dtypes.float8_e4m3)
        anc[:, 0:RPC] = anc_r[0:128]
        anc[:, RPC:2 * RPC] = anc_r[128:256]
        head = np.concatenate([anc, ct_in[:, 0:2048]], axis=1)
        fa_r = cfb[rows, :]                                # [RPC, D]
        fa = np.empty((128, RC * D), dtype=ml_dtypes.bfloat16)
        for rc in range(RC):
            fa[:, rc * D:(rc + 1) * D] = fa_r[rc * 128:(rc + 1) * 128]
        ug_vals = (1.0 - GAMMA) * u_np[idx[c * SPC:(c + 1) * SPC]]
        ug = np.ascontiguousarray(ug_vals.reshape(2, 128).T)  # [128, 2]
        in_maps.append({"head8": head, "fa": fa, "ug": ug, "ct8": ct_in,
                        "f8": f8_in})
    return in_maps


def _run(in_maps, trace=False, **kw):
    from concourse.bass_utils import run_bass_kernel_spmd

    nc = _get_module()
    return run_bass_kernel_spmd(
        nc, in_maps, core_ids=list(range(NCORES)), trace=trace, **kw
    )


def kernel(index, features, u):
    in_maps = _prep_inputs(index, features, u)
    res = _run(in_maps)
    total = 0.0
    for c in range(NCORES):
        total += np.asarray(res.results[c]["loss_rows"], dtype=np.float64).sum()
    return np.float32(total / N)
